# revision 1
# baseline (speedup 1.0000x reference)
"""Trainium2 Bass kernel for nn_Correlation (stereo cost volume).

  out[b, d, h, w] = mean_c( x[b,c,h,w] * y[b,c,h,w-d] ),  w >= d else 0
  B=8, C=32, H=256, W=512, D=48  (maxdisp=48)

Sharding: data-parallel over batch B across the 8 NeuronCores (one batch
element per core).  Each core computes its full [D, H, W] cost volume.

Per-core algorithm (fp32 compute, fp16 staging/output storage):
  - x/y rows are staged in SBUF in two 32-partition slabs (partitions
    0-31 and 64-95) so consecutive matmuls alternate PE row groups and
    LDWEIGHTS overlaps the running matmul.  y rows are stored
    contiguously with a 47-col lead so a single DMA per slab loads all
    G rows (windows that read across row boundaries only feed the w<d
    zone, which is zeroed later).
  - Per (h, 128-col w-tile): one PE matmul, K=C=32, stationary = X
    columns [32,128], moving = Y window [32,175].  psum[j, u] =
    <x_col(w0+j), y_col(w0+u-47)>, so the 48 outputs of column j sit on
    the diagonal u = j..j+47 (d = j+47-u).
  - DVE/ACT copies scale psum by 1/32 (the exact mean) into an SBUF
    G tile stored as fp16 (halves all downstream traffic; ~5e-4 rel
    error, values are O(1) means of N(0,1) products so no overflow);
    the w<d zone (cols 0:47 of w-tile 0) is zeroed.
  - G is dumped contiguously to a DRAM scratch, and a skewed
    DRAM->DRAM DMA (DRAM is linear, so arbitrary strides are legal -
    SBUF-side skewed access patterns mis-lower in the DGE descriptor
    generation, resetting the per-partition byte skew every 4
    partitions) walks the band diagonals straight into the output in
    [h, w, d_rev] layout with fully contiguous 98KB writes per row.
  - The host casts back to fp32, reverses d and transposes to
    [d, h, w].
"""

import sys

sys.path.insert(0, "/opt/trn_rl_repo")

import numpy as np
from contextlib import ExitStack

import concourse.bass as bass
import concourse.tile as tile
from concourse import mybir
from concourse import bass_utils

B = 8
C = 32
H = 256
W = 512
D = 48
NW = W // 128           # 4 w-tiles per row
MMN = 128 + D - 1       # 175 moving columns per matmul
LEAD = D - 1            # 47
GW = NW * MMN           # 700 G cols per h
G = 16                  # h rows per slab per iteration


def _split_waits(nc, max_waits=1):
    """Walrus codegen accepts at most ONE sync wait per instruction; Tile
    attaches several.  Split extra waits onto preceding NoOps on the same
    engine queue (dispatch is in-order, waits gate dispatch)."""
    for fn in nc.m.functions:
        for blk in fn.blocks:
            newl = []
            changed = False
            for inst in blk.instructions:
                si = getattr(inst, "sync_info", None)
                ow = list(si.on_wait) if si is not None and si.on_wait else []
                if len(ow) > max_waits and inst.engine is not None:
                    for k, wcond in enumerate(ow[:-max_waits]):
                        newl.append(mybir.InstNoOp(
                            name=f"{inst.name}w{k}",
                            engine=inst.engine,
                            sync_info=mybir.SyncInfo(on_wait=[wcond],
                                                     on_update=[]),
                        ))
                    inst.sync_info = mybir.SyncInfo(
                        on_wait=ow[-max_waits:],
                        on_update=list(si.on_update) if si.on_update else [])
                    changed = True
                newl.append(inst)
            if changed:
                blk.instructions = newl


def _emit_body(ctx, tc, x_ap, y_ap, o_ap, act_frac=0.34):
    nc = tc.nc
    n_iter = H // (2 * G)
    o_t = o_ap.tensor
    yflat = y_ap.rearrange("c h w -> c (h w)")

    # DRAM scratch: one [128, 2*GW] block per h-pair
    gd = nc.dram_tensor("gd", [(H // 2) * 128 * 2 * GW], mybir.dt.float16,
                        kind="Internal")

    xpool = ctx.enter_context(tc.tile_pool(name="xp", bufs=2))
    ypool = ctx.enter_context(tc.tile_pool(name="yp", bufs=2))
    gpool = ctx.enter_context(tc.tile_pool(name="gp", bufs=3))
    ppool = ctx.enter_context(tc.tile_pool(name="pp", bufs=6, space="PSUM"))

    inv_c = 1.0 / C
    hcount = 0

    for it in range(n_iter):
        h0 = it * 2 * G
        xt = xpool.tile([128, G * W], mybir.dt.float32, name=f"xt{it}", tag="xt")
        yt = ypool.tile([128, LEAD + G * W], mybir.dt.float32,
                        name=f"yt{it}", tag="yt")

        nc.sync.dma_start(xt[0:C, :], x_ap[:, h0:h0 + G, :])
        nc.sync.dma_start(xt[64:64 + C, :], x_ap[:, h0 + G:h0 + 2 * G, :])
        if it == 0:
            # no rows before row 0: lead cols stay unloaded; the very first
            # w-tile uses a shrunk moving window instead
            nc.sync.dma_start(yt[0:C, LEAD:], yflat[:, 0:G * W])
        else:
            nc.sync.dma_start(yt[0:C, :], yflat[:, h0 * W - LEAD:(h0 + G) * W])
        nc.sync.dma_start(yt[64:64 + C, :],
                          yflat[:, (h0 + G) * W - LEAD:(h0 + 2 * G) * W])

        for g in range(G):
            hs = (h0 + g, h0 + G + g)
            bases = (0, 64)
            gt = gpool.tile([128, 2 * GW], mybir.dt.float16,
                            name=f"gt{it}_{g}", tag="gt")
            psums = []
            for half in range(NW // 2):           # psum pair = 2 w-tiles
                ps = [
                    ppool.tile([128, 2 * MMN], mybir.dt.float32,
                               name=f"ps{it}_{g}_{half}_{s}", tag="ps",
                               padded_shape=[128, 512])
                    for s in range(2)
                ]
                for wsub in range(2):
                    wt = half * 2 + wsub
                    for s in range(2):
                        base = bases[s]
                        lhs = xt[base:base + C,
                                 g * W + wt * 128: g * W + wt * 128 + 128]
                        lo = LEAD if (it == 0 and g == 0 and s == 0
                                      and wt == 0) else 0
                        rhs = yt[base:base + C,
                                 g * W + wt * 128 + lo: g * W + wt * 128 + MMN]
                        nc.tensor.matmul(
                            ps[s][:, wsub * MMN + lo:(wsub + 1) * MMN],
                            lhs, rhs, start=True, stop=True)
                psums.append(ps)

            for s in range(2):
                for half in range(NW // 2):
                    lo = LEAD if (it == 0 and g == 0 and s == 0
                                  and half == 0) else 0
                    dst_sl = gt[:, s * GW + half * 2 * MMN + lo:
                                s * GW + (half + 1) * 2 * MMN]
                    src_sl = psums[half][s][:, lo:]
                    if (hcount % 100) < act_frac * 100:
                        nc.scalar.mul(dst_sl, src_sl, inv_c)
                    else:
                        nc.vector.tensor_scalar_mul(dst_sl, src_sl, inv_c)
                # zero the w<d zone (read from left of the row start)
                nc.vector.memset(gt[:, s * GW:s * GW + LEAD], 0.0)
                hcount += 1

            # dump the h-pair G to DRAM scratch (contiguous 717KB)
            pc = it * G + g
            dmp = bass.AP(gd, pc * 128 * 2 * GW, [[2 * GW, 128], [1, 2 * GW]])
            nc.sync.dma_start(dmp, gt[:, :])
            # skewed extraction per h: band diagonals -> [h, w, d_rev]
            # (all DMAs stay on the SP HWDGE ring: moving any to the ACT
            # ring serializes with the scalar-engine psum drains and
            # measured 27% slower)
            for s in range(2):
                h = hs[s]
                src = bass.AP(gd, pc * 128 * 2 * GW + s * GW,
                              [[2 * GW + 1, 128], [MMN, NW], [1, D]])
                dst = bass.AP(o_t, h * W * D,
                              [[D, 128], [128 * D, NW], [1, D]])
                nc.sync.dma_start(dst, src)


def _build_kernel():
    nc = bass.Bass(trn_type="TRN2", target_bir_lowering=False)
    x_d = nc.dram_tensor("x", [C, H, W], mybir.dt.float32, kind="ExternalInput")
    y_d = nc.dram_tensor("y", [C, H, W], mybir.dt.float32, kind="ExternalInput")
    o_d = nc.dram_tensor("o", [H, W, D], mybir.dt.float16,
                          kind="ExternalOutput")
    with ExitStack() as ctx:
        tc = ctx.enter_context(tile.TileContext(nc))
        _emit_body(ctx, tc, x_d.ap(), y_d.ap(), o_d.ap())
    _split_waits(nc)
    return nc


_NC_CACHE = None


def _get_nc():
    global _NC_CACHE
    if _NC_CACHE is None:
        _NC_CACHE = _build_kernel()
    return _NC_CACHE


def kernel(x: np.ndarray, y: np.ndarray, maxdisp=48) -> np.ndarray:
    assert int(maxdisp) == D
    x = np.ascontiguousarray(np.asarray(x, dtype=np.float32))
    y = np.ascontiguousarray(np.asarray(y, dtype=np.float32))
    assert x.shape == (B, C, H, W) and y.shape == (B, C, H, W)

    nc = _get_nc()
    in_maps = [{"x": x[b], "y": y[b]} for b in range(B)]
    res = bass_utils.run_bass_kernel_spmd(nc, in_maps, core_ids=list(range(B)))

    out = np.empty((B, D, H, W), dtype=np.float32)
    for b in range(B):
        ob = np.asarray(res.results[b]["o"], dtype=np.float32)
        out[b] = ob[:, :, ::-1].transpose(2, 0, 1)   # undo d reversal
    return out


if __name__ == "__main__":
    rng = np.random.default_rng(0)
    x = rng.standard_normal((B, C, H, W), dtype=np.float32)
    y = rng.standard_normal((B, C, H, W), dtype=np.float32)
    out = kernel(x=x, y=y, maxdisp=D)
    print("kernel output:", out.shape, out.dtype)



# revision 3
# speedup vs baseline: 2.3207x; 2.3207x over previous
"""Trainium2 Bass kernel for nn_Correlation (stereo cost volume).

  out[b, d, h, w] = mean_c( x[b,c,h,w] * y[b,c,h,w-d] ),  w >= d else 0
  B=8, C=32, H=256, W=512, D=48  (maxdisp=48)

Sharding: data-parallel over batch B across the 8 NeuronCores (one batch
element per core).  Each core computes its full [D, H, W] cost volume.

Per-core algorithm (fp16 inputs, fp32 psum, fp16 output):
  - Host pre-casts x/y to fp16 (halves input HBM traffic; PE runs fp16 at
    full rate and accumulates fp32, so rel-err stays ~1e-3).
  - y rows are staged with a 47-col zero gap before each row and 1 zero
    after (row pitch 560), so out-of-range disparity reads hit zeros and
    the w<d region needs no masking anywhere downstream.
  - Per (h, 128-col w-tile): TWO M=64 matmuls sharing the band structure:
      c=0: stationary x[w0:w0+64]    -> psum parts  0:64,  moving y[w0-47 .. w0+65)
      c=1: stationary x[w0+64:w0+128]-> psum parts 64:128, moving y[w0+17 .. w0+129)
    Each is K=32, N=112.  The 48 valid outputs of every partition j land
    on the diagonal q = j..j+47 of its own 112-col block, so the whole
    valid band for 2 w-tiles is a dense [128, 224] psum rectangle
    (2.33x smaller than the naive [128, 350] full-window psum).
    tile_position is auto-derived (row = lhsT slab 0/64, col = psum 0/64);
    emission alternates slab row-groups so LDWEIGHTS overlaps matmuls.
  - DVE/ACT copies scale the [128,224] rectangles by 1/32 into a fp16 G
    tile [128, 2, 448] (h-pair), which is DMA'd straight to the DRAM
    output in band layout (1792B-contiguous descriptors).  No DRAM->DRAM
    skew pass, no scratch round trip.
  - The host un-skews the diagonals with a strided numpy view and
    assembles [B, D, H, W] fp32.
"""

import sys

sys.path.insert(0, "/opt/trn_rl_repo")

import numpy as np
from contextlib import ExitStack

import concourse.bass as bass
import concourse.tile as tile
from concourse import mybir
from concourse import bass_utils

B = 8
C = 32
H = 256
W = 512
D = 48
LEAD = D - 1            # 47
WP = W + LEAD + 1       # 560: [47 zeros][512 data][1 zero] per staged y row
NW = W // 128           # 4 w-tiles per row
MMN = 112               # moving cols per half-tile matmul (64 + 47, padded to 112)
G = 16                  # h rows per slab per iteration
NG = H // 2             # 128 h-pair groups (output is g-major)


def _split_waits(nc, max_waits=1):
    """Walrus codegen accepts at most ONE sync wait per instruction; Tile
    attaches several.  Split extra waits onto preceding NoOps on the same
    engine queue (dispatch is in-order, waits gate dispatch)."""
    for fn in nc.m.functions:
        for blk in fn.blocks:
            newl = []
            changed = False
            for inst in blk.instructions:
                si = getattr(inst, "sync_info", None)
                ow = list(si.on_wait) if si is not None and si.on_wait else []
                if len(ow) > max_waits and inst.engine is not None:
                    for k, wcond in enumerate(ow[:-max_waits]):
                        newl.append(mybir.InstNoOp(
                            name=f"{inst.name}w{k}",
                            engine=inst.engine,
                            sync_info=mybir.SyncInfo(on_wait=[wcond],
                                                     on_update=[]),
                        ))
                    inst.sync_info = mybir.SyncInfo(
                        on_wait=ow[-max_waits:],
                        on_update=list(si.on_update) if si.on_update else [])
                    changed = True
                newl.append(inst)
            if changed:
                blk.instructions = newl


def _emit_body(ctx, tc, x_ap, y_ap, o_ap):
    nc = tc.nc
    n_iter = H // (2 * G)
    o_t = o_ap.tensor
    inv_c = 1.0 / C

    xpool = ctx.enter_context(tc.tile_pool(name="xp", bufs=2))
    ypool = ctx.enter_context(tc.tile_pool(name="yp", bufs=2))
    gpool = ctx.enter_context(tc.tile_pool(name="gp", bufs=4))
    ppool = ctx.enter_context(tc.tile_pool(name="pp", bufs=8, space="PSUM"))

    for it in range(n_iter):
        h0 = it * 2 * G
        # x slab: parts 0:32 rows h0..h0+G, parts 64:96 rows h0+G..h0+2G
        xt = xpool.tile([128, G, W], mybir.dt.float16, name=f"xt{it}", tag="xt")
        # y slab, row pitch 560 with leading 47 + trailing 1 zero cols
        yt = ypool.tile([128, G, WP], mybir.dt.float16, name=f"yt{it}", tag="yt")

        nc.sync.dma_start(xt[0:C, :, :], x_ap[:, h0:h0 + G, :])
        nc.sync.dma_start(xt[64:64 + C, :, :], x_ap[:, h0 + G:h0 + 2 * G, :])
        nc.sync.dma_start(yt[0:C, :, LEAD:LEAD + W], y_ap[:, h0:h0 + G, :])
        nc.sync.dma_start(yt[64:64 + C, :, LEAD:LEAD + W],
                          y_ap[:, h0 + G:h0 + 2 * G, :])
        nc.vector.memset(yt[0:C, :, 0:LEAD], 0.0)
        nc.vector.memset(yt[64:64 + C, :, 0:LEAD], 0.0)
        nc.vector.memset(yt[0:C, :, WP - 1:WP], 0.0)
        nc.vector.memset(yt[64:64 + C, :, WP - 1:WP], 0.0)

        for g in range(G):
            gt = gpool.tile([128, 2, NW * MMN], mybir.dt.float16,
                            name=f"gt{it}_{g}", tag="gt")
            # 4 psum tiles: (hh, pair); each [128, 224] = 2 w-tiles of band
            ps = [[ppool.tile([128, 2 * MMN], mybir.dt.float32,
                              name=f"ps{it}_{g}_{hh}_{pr}", tag="ps",
                              padded_shape=[128, 256])
                   for pr in range(2)] for hh in range(2)]
            # emission alternates hh (slab base 0/64) every matmul so the
            # next LDWEIGHTS row-group differs from the running matmul's
            for pr in range(2):
                for tp in range(2):          # w-tile within pair
                    w0 = (pr * 2 + tp) * 128
                    for c in range(2):       # 64-part psum half
                        for hh in range(2):
                            base = 64 * hh
                            lhsT = xt[base:base + C, g,
                                      w0 + 64 * c:w0 + 64 * c + 64]
                            rhs = yt[base:base + C, g,
                                     w0 + 64 * c:w0 + 64 * c + MMN]
                            dst = ps[hh][pr][64 * c:64 * c + 64,
                                             tp * MMN:(tp + 1) * MMN]
                            nc.tensor.matmul(dst, lhsT, rhs,
                                             start=True, stop=True)
            for hh in range(2):
                for pr in range(2):
                    dst = gt[:, hh, pr * 2 * MMN:(pr + 1) * 2 * MMN]
                    if hh == 0:
                        nc.vector.tensor_scalar_mul(dst, ps[hh][pr][:, :], inv_c)
                    else:
                        nc.scalar.mul(dst, ps[hh][pr][:, :], inv_c)

            # band-layout output: [g][p][hh][448] fp16, 1792B per (g,p)
            gabs = it * G + g
            dstd = bass.AP(o_t, gabs * 128 * 2 * 448,
                           [[2 * 448, 128], [448, 2], [1, 448]])
            nc.sync.dma_start(dstd, gt[:, :, :])


def _build_kernel():
    nc = bass.Bass(trn_type="TRN2", target_bir_lowering=False)
    x_d = nc.dram_tensor("x", [C, H, W], mybir.dt.float16, kind="ExternalInput")
    y_d = nc.dram_tensor("y", [C, H, W], mybir.dt.float16, kind="ExternalInput")
    o_d = nc.dram_tensor("o", [NG * 128 * 2 * 448], mybir.dt.float16,
                         kind="ExternalOutput")
    with ExitStack() as ctx:
        tc = ctx.enter_context(tile.TileContext(nc))
        _emit_body(ctx, tc, x_d.ap(), y_d.ap(), o_d.ap())
    _split_waits(nc)
    return nc


_NC_CACHE = None


def _get_nc():
    global _NC_CACHE
    if _NC_CACHE is None:
        _NC_CACHE = _build_kernel()
    return _NC_CACHE


def _unskew(o_flat: np.ndarray) -> np.ndarray:
    """Band layout [g(128), p(128), hh(2), q(448)] fp16 -> [D, H, W] fp32.

    q = pr*224 + tp*112 + qq;  p = pc*64 + r;  w = (2*pr+tp)*128 + pc*64 + r;
    h = it*32 + hh*16 + g2 (g = it*16 + g2);  value at qq = r + (47 - d).
    """
    R = o_flat.reshape(8, 16, 2, 64, 2, 2, 2, MMN)  # it,g2,pc,r,hh,pr,tp,qq
    s = R.strides
    V = np.lib.stride_tricks.as_strided(
        R, shape=(8, 16, 2, 64, 2, 2, 2, D),
        strides=(s[0], s[1], s[2], s[3] + s[7], s[4], s[5], s[6], s[7]))
    X = V.astype(np.float32)  # gather along the fast axis, then view-transpose
    # axes: it,g2,pc,r,hh,pr,tp,k  ->  d(=47-k), h(it,hh,g2), w(pr,tp,pc,r)
    Xf = X[..., ::-1]
    out = Xf.transpose(7, 0, 4, 1, 5, 6, 2, 3).reshape(D, H, W)
    return out


def kernel(x: np.ndarray, y: np.ndarray, maxdisp=48) -> np.ndarray:
    assert int(maxdisp) == D
    x = np.asarray(x)
    y = np.asarray(y)
    assert x.shape == (B, C, H, W) and y.shape == (B, C, H, W)
    x16 = np.ascontiguousarray(x.astype(np.float16))
    y16 = np.ascontiguousarray(y.astype(np.float16))

    nc = _get_nc()
    in_maps = [{"x": x16[b], "y": y16[b]} for b in range(B)]
    res = bass_utils.run_bass_kernel_spmd(nc, in_maps, core_ids=list(range(B)))

    out = np.empty((B, D, H, W), dtype=np.float32)
    for b in range(B):
        out[b] = _unskew(np.asarray(res.results[b]["o"]))
    return out


if __name__ == "__main__":
    rng = np.random.default_rng(0)
    x = rng.standard_normal((B, C, H, W), dtype=np.float32)
    y = rng.standard_normal((B, C, H, W), dtype=np.float32)
    out = kernel(x=x, y=y, maxdisp=D)
    print("kernel output:", out.shape, out.dtype)
